# revision 1
# baseline (speedup 1.0000x reference)
"""Trainium2 Bass kernel for nn_EqLayerNodeAttr (gnn message passing).

Strategy:
  - Edges sharded across 8 cores by whole destination-node (col) groups, so
    each core owns a disjoint set of output rows -> no collectives.
  - Within a core, edges are packed into tiles of <=512 edges covering <=64
    distinct destination nodes.  Per tile:
      * src node rows gathered with one multi-offset indirect DMA (bf16 table)
      * dst node rows: the <=64 distinct rows are gathered once ("window"),
        then expanded per-edge with a one-hot matmul on the PE
      * per-edge 2x2 rotations on DVE with broadcast access patterns
      * features transposed to [feat, edge] layout via PE transposes
      * 608->256->192 MLP as bf16 matmuls with fp32 PSUM accumulation
      * messages rotated back per edge, then segment-summed over the tile's
        <=64 destinations with a one-hot matmul and written to the output
        rows with an indirect scatter DMA (each dst row written exactly once
        globally -> no read-modify-write races).
"""

import numpy as np
import ml_dtypes

# ---- problem constants (hardcoded per contract) ----
N = 10000
E = 160000
L = 4
NS, NSA = 64, 16
NR, NRA = 16, 8
DIST = 64
HID = 256
SCAL = NS + NSA            # 80
NREP = NR + NRA            # 24
ROTF = NREP * 2 * L        # 192
FEAT = SCAL + ROTF         # 272
ROTD = ROTF * 2            # 384, l-duplicated rot features (j,k,m,l)
FEATD = SCAL + ROTD        # 464, node table row with dup rot part
DIN = 2 * FEAT + DIST      # 608
DOUT = NS + NR * 2 * L     # 192
DOUTD = NS + NR * L * 4    # 320, MLP2 out with dup rot part (j,k,m,l)
NCORES = 8

TP = 512                   # edges per tile
SUB = 128                  # edges per sub-tile
NSUBT = TP // SUB          # 4
W = 64                     # max distinct destination nodes per tile
NACC = N + W               # junk rows N..N+W-1 absorb padding writes
MW = 73                    # packed metadata words per lane

BF16 = ml_dtypes.bfloat16

# K-chunks of the MLP input (W1 rows reordered to match, see _w1_chunks):
#  c0: dst_rot[0:128]            (featT block 0)
#  c1: dst_rot[128:192] p0:64  | src_rot[128:192] p64:128   (featT block 1)
#  c2: src_rot[0:128]            (featT block 2)
#  c3: dst_scal[0:80]            (sdst tile)
#  c4: src_scal[0:80]            (ssrc tile)
#  c5: dist[0:64]                (dist tile)
KC = [128, 128, 128, SCAL, SCAL, DIST]


def _w1_chunks():
    dst_scal = np.arange(0, 80)
    dst_rot = np.arange(80, 272)
    src_scal = np.arange(272, 352)
    src_rot = np.arange(352, 544)
    dist = np.arange(544, 608)
    return [
        dst_rot[0:128],
        np.concatenate([dst_rot[128:192], src_rot[128:192]]),
        src_rot[0:128],
        dst_scal,
        src_scal,
        dist,
    ]


# --------------------------------------------------------------------------
# host-side sharding / tiling
# --------------------------------------------------------------------------

def _shard_and_tile(row, col):
    """Group edges by destination col; split whole cols across 8 cores with
    balanced edge counts; pack each core's cols into (<=TP edges, <=W cols)
    tiles."""
    order = np.argsort(col, kind="stable")
    scol = col[order]
    uniq, starts = np.unique(scol, return_index=True)
    starts = np.append(starts, len(scol))

    per_core_tiles = [[] for _ in range(NCORES)]
    core_cols = [[] for _ in range(NCORES)]
    target = len(scol) / NCORES
    ci = 0
    for ui in range(len(uniq)):
        lo = starts[ui]
        while ci < NCORES - 1 and lo >= (ci + 1) * target:
            ci += 1
        core_cols[ci].append(ui)

    for c in range(NCORES):
        tiles = []
        cur_e, cur_c = [], []
        for ui in core_cols[c]:
            lo, hi = starts[ui], starts[ui + 1]
            deg = hi - lo
            if deg > TP:
                raise ValueError("col degree exceeds tile capacity")
            if cur_e and (len(cur_e) + deg > TP or len(cur_c) + 1 > W):
                tiles.append((np.array(cur_e, np.int64), np.array(cur_c, np.int64)))
                cur_e, cur_c = [], []
            cur_e.extend(order[lo:hi].tolist())
            cur_c.append(int(uniq[ui]))
        if cur_e:
            tiles.append((np.array(cur_e, np.int64), np.array(cur_c, np.int64)))
        per_core_tiles[c] = tiles
    return per_core_tiles


def _host_prep(inputs):
    x_scalar = np.asarray(inputs["x_scalar"], np.float32)
    x_rot = np.asarray(inputs["x_rot"], np.float32)
    na_scalar = np.asarray(inputs["na_scalar"], np.float32)
    na_rot = np.asarray(inputs["na_rot"], np.float32)
    edge_index = np.asarray(inputs["edge_index"])
    dist_emb = np.asarray(inputs["dist_emb"], np.float32)
    rot = np.asarray(inputs["rot"], np.float32)
    W1 = np.asarray(inputs["W1"], np.float32)
    b1 = np.asarray(inputs["b1"], np.float32)
    W2 = np.asarray(inputs["W2"], np.float32)
    b2 = np.asarray(inputs["b2"], np.float32)

    row = edge_index[0].astype(np.int64)
    col = edge_index[1].astype(np.int64)

    # node table rows: [scal 80 | xr dup over l, order (j,k,m,l), 384]
    xs = np.concatenate([x_scalar, na_scalar], axis=1)                  # [N, 80]
    xr3 = np.concatenate([x_rot, na_rot], axis=1).reshape(N, NREP, L, 2)
    xr_dup = np.repeat(xr3[..., None], 2, axis=-1).reshape(N, ROTD)
    nodes = np.zeros((NACC, FEATD), np.float32)
    nodes[:N] = np.concatenate([xs, xr_dup], axis=1)
    nodes_bf16 = nodes.astype(BF16)

    per_core_tiles = _shard_and_tile(row, col)
    T = max(len(t) for t in per_core_tiles)

    W1c = np.zeros((6, 128, HID), np.float32)
    for c, idx in enumerate(_w1_chunks()):
        W1c[c, : len(idx)] = W1[idx]
    W1c = W1c.astype(BF16)
    # W2 cols: [scal 64 | dup over l, order (j,k,m,l), 256]
    W2r = W2[:, NS:].reshape(HID, NR, L, 2)
    W2d = np.concatenate(
        [W2[:, :NS], np.repeat(W2r[..., None], 2, axis=-1).reshape(HID, 256)],
        axis=1,
    )
    W2c = W2d.reshape(2, 128, DOUTD).astype(BF16)
    b1c = b1.reshape(2, 128).T.astype(np.float32).copy()   # [128, 2]

    # per-edge rote arrangements (bf16):
    #  fwd:  value rot[k,l,m] stored at (k,m,l)  -> transpose last two axes
    #  back: value rot[k,m,l] stored at (k,m,l)  -> natural order
    rot_fwd = np.ascontiguousarray(rot.transpose(0, 1, 3, 2)).reshape(-1, 16)
    rot_back = rot.reshape(-1, 16)

    per_core_inputs = []
    for c in range(NCORES):
        tiles = per_core_tiles[c]
        # packed per-lane metadata words:
        #  0:4 ridx | 4:8 crel | 8:40 rote_fwd (4 subs x 16 bf16)
        #  40:72 rote_back | 72 winrows (lanes 0..63)
        meta = np.zeros((T, SUB, MW), np.int32)
        meta[:, :, 4:8] = 127          # crel padding -> no onehot match
        dist = np.zeros((T, DIST, TP), BF16)
        rf_bf = np.zeros((T, SUB, NSUBT * 16), BF16)
        rb_bf = np.zeros((T, SUB, NSUBT * 16), BF16)
        winrows = np.tile(np.arange(W, dtype=np.int32) + N, (T, 1))
        for t in range(T):
            if t >= len(tiles):
                continue
            eids, cols = tiles[t]
            ne, ncol = len(eids), len(cols)
            winrows[t, :ncol] = cols.astype(np.int32)
            slot = np.arange(ne)
            lane, s = slot % SUB, slot // SUB
            m = meta[t]
            m[lane, s] = row[eids].astype(np.int32)
            m[lane, 4 + s] = np.searchsorted(cols, col[eids]).astype(np.int32)
            cidx = (s * 16)[:, None] + np.arange(16)
            rf_bf[t, lane[:, None], cidx] = rot_fwd[eids].astype(BF16)
            rb_bf[t, lane[:, None], cidx] = rot_back[eids].astype(BF16)
            dist[t, :, :ne] = dist_emb[eids].T.astype(BF16)

        def pack(bf):
            u = bf.view(np.uint16).reshape(T, SUB, 32, 2).astype(np.uint32)
            return (u[..., 0] | (u[..., 1] << 16)).view(np.int32)

        meta[:, :, 8:40] = pack(rf_bf)
        meta[:, :, 40:72] = pack(rb_bf)
        meta[:, :W, 72] = winrows
        per_core_inputs.append(
            dict(
                nodes=nodes_bf16,
                W1c=W1c,
                W2c=W2c,
                b1c=b1c,
                meta=meta,
                dist=dist,
            )
        )

    meta_info = dict(per_core_tiles=per_core_tiles, row=row, col=col,
                     rot=rot, b2=b2)
    return per_core_inputs, T, meta_info


def _assemble(results, meta):
    col = meta["col"]
    deg = np.bincount(col, minlength=N)
    out = np.zeros((N, DOUT), np.float32)
    for c, tiles in enumerate(meta["per_core_tiles"]):
        acc = results[c]["acc"]
        for eids, cols in tiles:
            out[cols] = acc[cols]
    out[deg == 0] = 0.0
    b2 = meta["b2"]
    if np.any(b2):
        out[:, :NS] += np.outer(deg, b2[:NS])
        b2r = b2[NS:].reshape(NR, L, 2)
        rot = meta["rot"]
        corr = np.einsum("jkm,ekml->ejkl", b2r, rot).reshape(E, NR * 2 * L)
        np.add.at(out[:, NS:], col, corr)
    return out


# --------------------------------------------------------------------------
# device program
# --------------------------------------------------------------------------

def _build_program(T):
    from concourse import bacc, mybir
    import concourse.tile as tile
    from concourse.bass import IndirectOffsetOnAxis
    from concourse.masks import make_identity

    f32 = mybir.dt.float32
    bf16 = mybir.dt.bfloat16
    i32 = mybir.dt.int32
    AL = mybir.AluOpType
    ACTF = mybir.ActivationFunctionType

    nc = bacc.Bacc("TRN2", target_bir_lowering=False, debug=False)

    d_nodes = nc.dram_tensor("nodes", [NACC, FEATD], bf16, kind="ExternalInput").ap()
    d_W1c = nc.dram_tensor("W1c", [6, 128, HID], bf16, kind="ExternalInput").ap()
    d_W2c = nc.dram_tensor("W2c", [2, 128, DOUTD], bf16, kind="ExternalInput").ap()
    d_b1c = nc.dram_tensor("b1c", [128, 2], f32, kind="ExternalInput").ap()
    d_meta = nc.dram_tensor("meta", [T, SUB, MW], i32, kind="ExternalInput").ap()
    d_dist = nc.dram_tensor("dist", [T, DIST, TP], bf16, kind="ExternalInput").ap()
    d_acc = nc.dram_tensor("acc", [NACC, DOUT], f32, kind="ExternalOutput").ap()

    with tile.TileContext(nc) as tc:
        with (
            tc.tile_pool(name="const", bufs=1) as cpool,
            tc.tile_pool(name="sb", bufs=3) as pool,
            tc.tile_pool(name="sb3", bufs=4) as pool3,
            tc.tile_pool(name="ph", bufs=2, space="PSUM") as pph,
            tc.tile_pool(name="po", bufs=1, space="PSUM") as ppo,
            tc.tile_pool(name="ptr", bufs=3, space="PSUM") as ptr,
            tc.tile_pool(name="px", bufs=1, space="PSUM") as px,
            tc.tile_pool(name="psc", bufs=1, space="PSUM") as psc,
        ):
            # ---- constants ----
            ident = cpool.tile([128, 128], bf16)
            make_identity(nc, ident[:])
            iota = cpool.tile([128, W], i32)
            nc.gpsimd.iota(iota[:], pattern=[[1, W]], base=0, channel_multiplier=0)
            w1sb = cpool.tile([128, 6 * HID], bf16)
            for c in range(6):
                nc.sync.dma_start(out=w1sb[:, c * HID:(c + 1) * HID], in_=d_W1c[c])
            w2sb = cpool.tile([128, 2 * DOUTD], bf16)
            for c in range(2):
                nc.sync.dma_start(
                    out=w2sb[:, c * DOUTD:(c + 1) * DOUTD], in_=d_W2c[c]
                )
            b1sb = cpool.tile([128, 2], f32)
            nc.sync.dma_start(out=b1sb[:], in_=d_b1c[:])

            def emit_front(t):
                # ---- per-tile loads ----
                meta = pool.tile([SUB, MW], i32)
                nc.sync.dma_start(out=meta[:], in_=d_meta[t])
                dist_sb = pool.tile([DIST, TP], bf16)
                nc.sync.dma_start(out=dist_sb[:], in_=d_dist[t])
                ridx = meta[:, 0:4]
                crel = meta[:, 4:8]
                rote_f = meta[:, 8:40].bitcast(bf16)     # [128, 64]
                rote_b = meta[:, 40:72].bitcast(bf16)    # [128, 64]
                winr = meta[0:W, 72:73]

                # ---- one-hots (independent of gathers) ----
                onehot_e = pool.tile([SUB, NSUBT * W], bf16)
                onehot_w = pool.tile([W, TP], bf16)
                p_oh = ptr.tile([W, TP], bf16, tag="ptrans")
                for s in range(NSUBT):
                    oh_e = onehot_e[:, s * W:(s + 1) * W]
                    nc.vector.tensor_tensor(
                        out=oh_e,
                        in0=crel[:, s:s + 1].to_broadcast([SUB, W]),
                        in1=iota[:, :],
                        op=AL.is_equal,
                    )
                    nc.tensor.transpose(
                        out=p_oh[:, s * SUB:(s + 1) * SUB], in_=oh_e,
                        identity=ident[:],
                    )
                nc.scalar.activation(out=onehot_w[:], in_=p_oh[:], func=ACTF.Copy)

                # ---- gathers ----
                win = pool.tile([W, FEATD], bf16)
                nc.gpsimd.indirect_dma_start(
                    out=win[:],
                    out_offset=None,
                    in_=d_nodes[:],
                    in_offset=IndirectOffsetOnAxis(ap=winr, axis=0),
                )
                src_g = pool.tile([SUB, NSUBT * FEATD], bf16)
                for s in range(NSUBT):
                    nc.gpsimd.indirect_dma_start(
                        out=src_g[:, s * FEATD:(s + 1) * FEATD],
                        out_offset=None,
                        in_=d_nodes[:],
                        in_offset=IndirectOffsetOnAxis(ap=ridx[:, s:s + 1], axis=0),
                    )

                featT = pool.tile([128, 3 * TP], bf16)
                sdst = pool.tile([SCAL, TP], bf16)
                ssrc = pool.tile([SCAL, TP], bf16)

                def fwd_rotate(dup_view, rote_ap, out_tile, eng):
                    """out[(j,k,l)] = sum_m dup[(j,k,m,l)] * rote[(k,m,l)],
                    with rote_fwd storing rot[k,l,m] at (k,m,l)."""
                    tmp = pool3.tile([SUB, ROTD], bf16, tag="tmprot")
                    r_b = (
                        rote_ap.rearrange("p (k q) -> p k q", k=L, q=4)
                        .unsqueeze(1)
                        .broadcast_to([SUB, NREP, L, 4])
                    )
                    eng.tensor_tensor(
                        out=tmp[:].rearrange("p (j k q) -> p j k q",
                                             j=NREP, k=L, q=4),
                        in0=dup_view,
                        in1=r_b,
                        op=AL.mult,
                    )
                    tv = tmp[:].rearrange("p (a m l) -> p a m l",
                                          a=NREP * L, m=2, l=2)
                    eng.tensor_tensor(
                        out=out_tile.rearrange("p (a l) -> p a l",
                                               a=NREP * L, l=2),
                        in0=tv[:, :, 0, :],
                        in1=tv[:, :, 1, :],
                        op=AL.add,
                    )

                for s in range(NSUBT):
                    cL = s * SUB
                    rfs = rote_f[:, s * 16:(s + 1) * 16]

                    # ---- dst rot features: expand + rotate ----
                    p_x1 = px.tile([SUB, ROTD], f32, tag="px")
                    nc.tensor.matmul(
                        out=p_x1[:],
                        lhsT=onehot_w[:, cL:cL + SUB],
                        rhs=win[:, SCAL:FEATD],
                        start=True,
                        stop=True,
                    )
                    dst_rot = pool3.tile([SUB, ROTF], bf16, tag="dstrot")
                    fwd_rotate(
                        p_x1[:].rearrange("p (j k q) -> p j k q",
                                          j=NREP, k=L, q=4),
                        rfs, dst_rot[:], nc.vector,
                    )

                    # ---- src rot features ----
                    sg = src_g[:, s * FEATD:(s + 1) * FEATD]
                    src_rot = pool3.tile([SUB, ROTF], bf16, tag="srcrot")
                    fwd_rotate(
                        sg[:, SCAL:FEATD].rearrange(
                            "p (j k q) -> p j k q", j=NREP, k=L, q=4
                        ),
                        rfs, src_rot[:], nc.vector,
                    )

                    # ---- transposes into chunk tiles ----
                    ptn = ptr.tile([128, 512], bf16, tag="ptrans")
                    nc.tensor.transpose(
                        out=ptn[:, 0:128], in_=dst_rot[:, 0:128], identity=ident[:]
                    )
                    nc.tensor.transpose(
                        out=ptn[0:64, 128:256], in_=dst_rot[:, 128:192],
                        identity=ident[:],
                    )
                    nc.tensor.transpose(
                        out=ptn[64:128, 128:256], in_=src_rot[:, 128:192],
                        identity=ident[:],
                    )
                    nc.tensor.transpose(
                        out=ptn[:, 256:384], in_=src_rot[:, 0:128], identity=ident[:]
                    )
                    nc.tensor.transpose(
                        out=ptn[0:SCAL, 384:512], in_=sg[:, 0:SCAL], identity=ident[:]
                    )
                    # merged copy of the three 128-part sections -> featT blocks
                    nc.scalar.activation(
                        out=featT[:].rearrange("p (c e) -> p c e", c=3, e=TP)[
                            :, :, cL:cL + SUB
                        ],
                        in_=ptn[:, 0:384].rearrange("p (c e) -> p c e", c=3, e=SUB),
                        func=ACTF.Copy,
                    )
                    nc.scalar.activation(
                        out=ssrc[:, cL:cL + SUB], in_=ptn[0:SCAL, 384:512],
                        func=ACTF.Copy,
                    )

                # ---- dst scalar expand (once per tile) ----
                p_x2 = pph.tile([SCAL, TP], f32, tag="ph")
                nc.tensor.matmul(
                    out=p_x2[:],
                    lhsT=win[:, 0:SCAL],
                    rhs=onehot_w[:],
                    start=True,
                    stop=True,
                )
                nc.scalar.activation(out=sdst[:], in_=p_x2[:], func=ACTF.Copy)

                # ---- MLP layer 1 + relu ----
                rhs_chunks = [
                    featT[:, 0:TP], featT[:, TP:2 * TP], featT[:, 2 * TP:3 * TP],
                    sdst[:], ssrc[:], dist_sb[:],
                ]
                hT = pool.tile([128, 2 * TP], bf16)
                for hh in range(2):
                    p_h = pph.tile([128, TP], f32, tag="ph")
                    for c in range(6):
                        nc.tensor.matmul(
                            out=p_h[:],
                            lhsT=w1sb[0:KC[c], c * HID + hh * 128:c * HID + (hh + 1) * 128],
                            rhs=rhs_chunks[c][0:KC[c], :],
                            start=(c == 0),
                            stop=(c == 5),
                        )
                    nc.scalar.activation(
                        out=hT[:, hh * TP:(hh + 1) * TP],
                        in_=p_h[:],
                        func=ACTF.Relu,
                        bias=b1sb[:, hh:hh + 1],
                    )

                # ---- MLP layer 2 (dup output cols, 3 partition chunks) ----
                msgT = []
                for dd, (d0, dw) in enumerate([(0, 128), (128, 128), (256, 64)]):
                    p_o = ppo.tile([dw, TP], f32, tag="po")
                    for hh in range(2):
                        nc.tensor.matmul(
                            out=p_o[:],
                            lhsT=w2sb[:, hh * DOUTD + d0:hh * DOUTD + d0 + dw],
                            rhs=hT[:, hh * TP:(hh + 1) * TP],
                            start=(hh == 0),
                            stop=(hh == 1),
                        )
                    mt = pool.tile([dw, TP], bf16, tag=f"msgT{dd}")
                    if dd == 0:
                        nc.vector.tensor_copy(out=mt[:], in_=p_o[:])
                    else:
                        nc.scalar.activation(out=mt[:], in_=p_o[:], func=ACTF.Copy)
                    msgT.append(mt)

                return dict(rote_b=rote_b, onehot_e=onehot_e,
                            winr=winr, msgT=msgT)

            def emit_back(st):
                rote_b = st["rote_b"]
                onehot_e = st["onehot_e"]
                winr = st["winr"]
                msgT = st["msgT"]
                # ---- back-rotation + scatter ----
                p_sc = psc.tile([W, DOUT], f32, tag="psc")
                for s in range(NSUBT):
                    cL = s * SUB
                    rbs = rote_b[:, s * 16:(s + 1) * 16]
                    p_m = ptr.tile([128, DOUTD], bf16, tag="ptrans")
                    nc.tensor.transpose(
                        out=p_m[:, 0:128], in_=msgT[0][:, cL:cL + SUB],
                        identity=ident[:],
                    )
                    nc.tensor.transpose(
                        out=p_m[:, 128:256], in_=msgT[1][:, cL:cL + SUB],
                        identity=ident[:],
                    )
                    nc.tensor.transpose(
                        out=p_m[:, 256:320], in_=msgT[2][:, cL:cL + SUB],
                        identity=ident[0:64, 0:64],
                    )
                    out_sb = pool3.tile([SUB, DOUT], bf16, tag="outsb")
                    nc.scalar.activation(out=out_sb[:, 0:NS], in_=p_m[:, 0:NS], func=ACTF.Copy)
                    # out[(j,k,l)] = sum_m msgdup[(j,k,m,l)] * rote_b[(k,m,l)]
                    tmpb = pool3.tile([SUB, 256], bf16, tag="tmpback")
                    r_b = (
                        rbs.rearrange("p (k q) -> p k q", k=L, q=4)
                        .unsqueeze(1)
                        .broadcast_to([SUB, NR, L, 4])
                    )
                    nc.vector.tensor_tensor(
                        out=tmpb[:].rearrange("p (j k q) -> p j k q",
                                              j=NR, k=L, q=4),
                        in0=p_m[:, NS:DOUTD].rearrange(
                            "p (j k q) -> p j k q", j=NR, k=L, q=4
                        ),
                        in1=r_b,
                        op=AL.mult,
                    )
                    tb = tmpb[:].rearrange("p (a m l) -> p a m l",
                                           a=NR * L, m=2, l=2)
                    nc.vector.tensor_tensor(
                        out=out_sb[:, NS:DOUT].rearrange(
                            "p (a l) -> p a l", a=NR * L, l=2
                        ),
                        in0=tb[:, :, 0, :],
                        in1=tb[:, :, 1, :],
                        op=AL.add,
                    )
                    nc.tensor.matmul(
                        out=p_sc[:],
                        lhsT=onehot_e[:, s * W:(s + 1) * W],
                        rhs=out_sb[:],
                        start=(s == 0),
                        stop=(s == NSUBT - 1),
                    )
                out_f = pool.tile([W, DOUT], f32)
                nc.scalar.activation(out=out_f[:], in_=p_sc[:], func=ACTF.Copy)
                nc.gpsimd.indirect_dma_start(
                    out=d_acc[:],
                    out_offset=IndirectOffsetOnAxis(ap=winr, axis=0),
                    in_=out_f[:],
                    in_offset=None,
                )

            # software pipeline: emit front(t+1) before back(t) so the
            # scheduler interleaves t+1's gathers/rotations with t's MLP
            st = emit_front(0)
            for t in range(1, T):
                st_next = emit_front(t)
                emit_back(st)
                st = st_next
            emit_back(st)

    nc.compile()
    return nc


_PROGRAM_CACHE = {}


def _get_program(T):
    if T not in _PROGRAM_CACHE:
        _PROGRAM_CACHE[T] = _build_program(T)
    return _PROGRAM_CACHE[T]


class _PjrtExec:
    """Persistent jitted SPMD executable for one Bass program (axon/PJRT)."""

    def __init__(self, nc):
        import jax
        from jax.sharding import Mesh, PartitionSpec
        from jax.experimental.shard_map import shard_map
        import concourse.mybir as mybir
        from concourse.bass2jax import (
            _bass_exec_p,
            install_neuronx_cc_hook,
            partition_id_tensor,
        )

        install_neuronx_cc_hook()
        self.nc = nc
        partition_name = (
            nc.partition_id_tensor.name if nc.partition_id_tensor else None
        )
        in_names, out_names, out_avals, zero_shapes = [], [], [], []
        for alloc in nc.m.functions[0].allocations:
            if not isinstance(alloc, mybir.MemoryLocationSet):
                continue
            name = alloc.memorylocations[0].name
            if alloc.kind == "ExternalInput":
                if name != partition_name:
                    in_names.append(name)
            elif alloc.kind == "ExternalOutput":
                shape = tuple(alloc.tensor_shape)
                dtype = mybir.dt.np(alloc.dtype)
                out_names.append(name)
                out_avals.append(jax.core.ShapedArray(shape, dtype))
                zero_shapes.append((shape, dtype))
        self.in_names = in_names
        self.out_names = out_names
        self.out_avals = out_avals
        self.zero_shapes = zero_shapes
        n_params, n_outs = len(in_names), len(out_names)
        all_names = in_names + out_names
        if partition_name is not None:
            all_names.append(partition_name)
        donate = tuple(range(n_params, n_params + n_outs))

        def _body(*args):
            operands = list(args)
            if partition_name is not None:
                operands.append(partition_id_tensor())
            outs = _bass_exec_p.bind(
                *operands,
                out_avals=tuple(out_avals),
                in_names=tuple(all_names),
                out_names=tuple(out_names),
                lowering_input_output_aliases=(),
                sim_require_finite=True,
                sim_require_nnan=True,
                nc=nc,
            )
            return tuple(outs)

        devices = jax.devices()[:NCORES]
        mesh = Mesh(np.asarray(devices), ("core",))
        self.fn = jax.jit(
            shard_map(
                _body,
                mesh=mesh,
                in_specs=(PartitionSpec("core"),) * (n_params + n_outs),
                out_specs=(PartitionSpec("core"),) * n_outs,
                check_rep=False,
            ),
            donate_argnums=donate,
            keep_unused=True,
        )

    def stage_inputs(self, per_core_inputs):
        import jax

        concat_in = [
            np.concatenate(
                [np.asarray(per_core_inputs[c][n]) for c in range(NCORES)], axis=0
            )
            for n in self.in_names
        ]
        return [jax.device_put(a) for a in concat_in]

    def fresh_zeros(self):
        return [
            np.zeros((NCORES * s[0], *s[1:]), d) for (s, d) in self.zero_shapes
        ]

    def run(self, staged, zeros):
        import jax

        outs = self.fn(*staged, *zeros)
        jax.block_until_ready(outs)
        return outs

    def results(self, outs):
        res = []
        for c in range(NCORES):
            res.append(
                {
                    n: np.asarray(outs[i]).reshape(
                        NCORES, *self.out_avals[i].shape
                    )[c]
                    for i, n in enumerate(self.out_names)
                }
            )
        return res


_EXEC_CACHE = {}


def _get_exec(T):
    if T not in _EXEC_CACHE:
        _EXEC_CACHE[T] = _PjrtExec(_get_program(T))
    return _EXEC_CACHE[T]


def kernel(**inputs):
    per_core_inputs, T, meta = _host_prep(inputs)
    ex = _get_exec(T)
    staged = ex.stage_inputs(per_core_inputs)
    outs = ex.run(staged, ex.fresh_zeros())
    return _assemble(ex.results(outs), meta)

